# revision 33
# baseline (speedup 1.0000x reference)
"""Trainium2 Bass kernel for nn_LlamaAttention_7352984010786.

Key insight: the reference's attention matrix is softmax(r @ r.T) where r is
the (input-independent) RoPE sinusoid table.  r_i . r_j = sum_d cos((i-j)*f_d)
is Toeplitz and decays so fast off-diagonal that after softmax the matrix is
numerically tridiagonal:

    probs[i,j] = c_|i-j| / Z_i,   c0 = 1, c1 = 3.6078e-05, c2 = 2.2e-16

So   out = ((v + c1*(v_shift_left + v_shift_right)) / Z) @ Wo.T,  v = x @ Wv.T

The band stencil is a sequence-space convolution and Wv acts on the feature
dim, so they commute:  band(x @ Wv.T) = band(x) @ Wv.T.  The band (and the
1/Z row normalization, folded into Wv with a 2-row edge fixup on x) is applied
ON THE HOST.  The device kernel is then two clean back-to-back matmuls per
core in bf16 (fp32 PSUM accumulation):

    y   = xb @ (Wv.T/Z)      [2048 x 768] @ [768 x 256]
    out = y @ Wo.T           [2048 x 256] @ [256 x 768]

Sharding: batch x seq chunks, 8 cores x 2048 rows.  All tensors are
pre-packed on the host into exact SBUF layout ([128 partitions, free]) so
every DMA moves multi-KB contiguous rows per partition at HBM line rate.
Per-core traffic ~7.1 MB bf16 (~20 us at 358 GB/s) ~= PE time (49k cycles
~= 21 us at 2.4 GHz): the roofline ridge.

Schedule notes (from perfetto traces): each dma_start costs ~0.65 us of
serial descriptor-gen on the issuing engine, so inputs are issued from Sync
and output DMAs from Scalar; the first weight/input chunks are split so the
PE starts ~1 us earlier; chunk sizes taper at the end to shrink the drain
tail; PSUM->SBUF output copies are paired across two PSUM banks to halve
instruction count.
"""

import os
import sys

import numpy as np

for _p in ("/opt/trn_rl_repo", os.path.expanduser("~/.axon_site/_ro/trn_rl_repo")):
    if os.path.isdir(_p) and _p not in sys.path:
        sys.path.insert(0, _p)

B, S, H, C = 2, 8192, 768, 256
THETA = 10000.0
NCORES = 8
CHUNK = S // 4      # 2048 seq rows per core; core k: batch k//4, quarter k%4
KH = H // 128       # 6 x-feature blocks (contraction of v-proj)
KC = C // 128       # 2 v-feature blocks (contraction of o-proj)
CHUNKS = [512, 512, 512, 512]               # per-core sequence chunking
OFFS = [0, 512, 1024, 1536]
NJ = len(CHUNKS)

_cache: dict = {}


def _band_constants():
    """c1 and the softmax row-normalizers, in fp64."""
    freqs = THETA ** (-np.arange(0, H, 2, dtype=np.float64) / H)
    dd = np.arange(S, dtype=np.float64)
    g = np.cos(np.outer(dd, freqs)).sum(1)
    e = np.exp(g - g[0])
    c1 = e[1]
    efull = np.concatenate([e[::-1], e[1:]])
    csum = np.concatenate([[0.0], np.cumsum(efull)])
    idx = np.arange(S)
    z = csum[idx + S] - csum[idx]  # Z_i = sum_j e(|i-j|)
    return c1, z


def _build_bass():
    import concourse.bass as bass
    import concourse.tile as tile
    from concourse import bacc, mybir

    f32 = mybir.dt.float32
    bf16 = mybir.dt.bfloat16

    nc = bacc.Bacc("TRN2", target_bir_lowering=False, debug=False,
                   num_devices=NCORES)

    xb_d = nc.dram_tensor("xb", [128, KH * CHUNK], bf16,
                          kind="ExternalInput").ap()
    wv_d = nc.dram_tensor("wv", [128, KH * C], bf16, kind="ExternalInput").ap()
    wo_d = nc.dram_tensor("wo", [128, KC * H], bf16, kind="ExternalInput").ap()
    out_d = nc.dram_tensor("out", [128, KH * CHUNK], bf16,
                           kind="ExternalOutput").ap()

    with tile.TileContext(nc) as tc:
        with (
            tc.tile_pool(name="const", bufs=1) as const_pool,
            tc.tile_pool(name="xin", bufs=NJ) as xin_pool,
            tc.tile_pool(name="y", bufs=1) as y_pool,
            tc.tile_pool(name="outs", bufs=2) as out_pool,
            tc.tile_pool(name="ps", bufs=2, space="PSUM") as ps_pool,
        ):
            # PE warmup fed from a memset tile (no DMA dependency): keeps
            # the tensor engine busy from ~7.5 us so the DVFS clock is
            # fully ramped when the first real matmul issues.  Uses the
            # psv pool (slot is recycled before the first real v-proj).
            wtile = const_pool.tile([128, 512], bf16, name="warm")
            nc.gpsimd.memset(wtile[:], 0.25)
            wps = ps_pool.tile([128, 512], f32, name="ps0")
            for _ in range(10):
                nc.tensor.matmul(wps[:], wtile[:, :128], wtile[:],
                                 start=True, stop=True)
            for _ in range(3):
                nc.tensor.matmul(wps[:, :128], wtile[:, :128],
                                 wtile[:, :128], start=True, stop=True)
            # weights split in k-halves so v-proj(0) can begin after only
            # half the weight bytes + half of chunk 0 have landed
            wv = [const_pool.tile([128, 3 * C], bf16, name=f"wv{i}")
                  for i in range(2)]
            wo = const_pool.tile([128, KC * H], bf16, name="wo")
            xb = [xin_pool.tile([128, KH * CHUNKS[j]], bf16, name=f"xb{j}")
                  for j in range(NJ)]

            # DMA issue order chosen so payloads land just-in-time
            nc.sync.dma_start(wv[0][:], wv_d[:, :3 * C])
            half0 = 3 * CHUNKS[0]
            nc.sync.dma_start(xb[0][:, :half0], xb_d[:, :half0])
            nc.sync.dma_start(xb[0][:, half0:], xb_d[:, half0:2 * half0])
            nc.sync.dma_start(wv[1][:], wv_d[:, 3 * C:])
            half1 = 3 * CHUNKS[1]
            base1 = KH * OFFS[1]
            nc.sync.dma_start(xb[1][:, :half1],
                              xb_d[:, base1:base1 + half1])
            nc.sync.dma_start(xb[1][:, half1:],
                              xb_d[:, base1 + half1:base1 + 2 * half1])
            nc.sync.dma_start(wo[:], wo_d)
            for j in range(2, NJ):
                nc.sync.dma_start(
                    xb[j][:],
                    xb_d[:, KH * OFFS[j]:KH * (OFFS[j] + CHUNKS[j])])

            yT = [y_pool.tile([128, CHUNK], bf16, name=f"y{cs}")
                  for cs in range(KC)]

            def wv_blk(k, cs):
                return wv[k // 3][:, (k % 3) * C + cs * 128:
                                  (k % 3) * C + (cs + 1) * 128]

            def vproj(j):
                L = CHUNKS[j]
                for cs in range(KC):
                    ps = ps_pool.tile([128, L], f32, name=f"ps{cs}")
                    for k in range(KH):
                        nc.tensor.matmul(
                            ps[:], wv_blk(k, cs),
                            xb[j][:, k * L:(k + 1) * L],
                            start=(k == 0), stop=(k == KH - 1),
                        )
                    dst = yT[cs][:, OFFS[j]:OFFS[j] + L]
                    if cs == 0:
                        nc.vector.tensor_copy(dst, ps[:])
                    else:
                        nc.scalar.copy(dst, ps[:])

            onalloc = [0]

            def oproj(j):
                L = CHUNKS[j]
                lo = KH * OFFS[j]
                last = (j == NJ - 1)
                ot = out_pool.tile([128, KH * L], bf16, tag="out",
                                   name=f"ot{j % 2}")
                for hh in range(KH):
                    ps = ps_pool.tile([128, L], f32,
                                      name=f"ps{onalloc[0] % 3}")
                    onalloc[0] += 1
                    for cs in range(KC):
                        nc.tensor.matmul(
                            ps[:],
                            wo[:, cs * H + hh * 128:cs * H + (hh + 1) * 128],
                            yT[cs][:, OFFS[j]:OFFS[j] + L],
                            start=(cs == 0), stop=(cs == KC - 1),
                        )
                    dst = ot[:, hh * L:(hh + 1) * L]
                    if last and hh >= 4:
                        # drain-critical pieces: copy AND issue the store
                        # from parallel DGE paths (ACT's own + idle Sync)
                        if hh == 4:
                            nc.scalar.copy(dst, ps[:])
                            nc.scalar.dma_start(
                                out_d[:, lo + 4 * L:lo + 5 * L], dst)
                        else:
                            nc.vector.tensor_copy(dst, ps[:])
                            nc.sync.dma_start(
                                out_d[:, lo + 5 * L:lo + 6 * L], dst)
                    elif hh % 2 == 0:
                        nc.vector.tensor_copy(dst, ps[:])
                    else:
                        nc.scalar.copy(dst, ps[:])
                        # store each third as soon as its copies land
                        nc.sync.dma_start(
                            out_d[:, lo + (hh - 1) * L:lo + (hh + 1) * L],
                            ot[:, (hh - 1) * L:(hh + 1) * L])

            vproj(0)
            for j in range(NJ):
                if j + 1 < NJ:
                    vproj(j + 1)
                oproj(j)

    nc.compile()
    return nc


def _get_nc():
    if "nc" not in _cache:
        _cache["nc"] = _build_bass()
    return _cache["nc"]


def kernel(**inputs) -> np.ndarray:
    out, _ = _run(inputs)
    return out


def _prep(inputs):
    import ml_dtypes
    bf16 = ml_dtypes.bfloat16

    x = np.asarray(inputs["x"], dtype=np.float32)
    Wv = np.asarray(inputs["Wv"], dtype=np.float32)
    Wo = np.asarray(inputs["Wo"], dtype=np.float32)

    c1, z = _band_constants()
    z_int = 1.0 + 2.0 * c1

    # band on x (commutes with the projections); 1/Z folded into Wv as
    # 1/z_int, with the two edge rows rescaled here to their true Z.
    xb = np.empty_like(x)
    xb[:, 1:-1, :] = x[:, 1:-1, :] + np.float32(c1) * (x[:, :-2, :]
                                                       + x[:, 2:, :])
    xb[:, 0, :] = (x[:, 0, :] + np.float32(c1) * x[:, 1, :]) \
        * np.float32(z_int / z[0])
    xb[:, -1, :] = (x[:, -1, :] + np.float32(c1) * x[:, -2, :]) \
        * np.float32(z_int / z[-1])
    xb = xb.astype(bf16)

    # SBUF-layout packing: wv[p, k, c] = Wv[c, k*128+p]/z_int
    wv_sb = np.ascontiguousarray(
        (Wv.T * np.float32(1.0 / z_int)).astype(bf16)
        .reshape(KH, 128, C).transpose(1, 0, 2).reshape(128, KH * C))
    # wo[p, s, h] = Wo[h, s*128+p]
    wo_sb = np.ascontiguousarray(
        Wo.T.astype(bf16).reshape(KC, 128, H).transpose(1, 0, 2)
        .reshape(128, KC * H))

    in_maps = []
    for core in range(NCORES):
        b, q = divmod(core, 4)
        blk = xb[b, q * CHUNK:(q + 1) * CHUNK, :]          # [2048, 768]
        # per chunk j: [L, 768] -> [128(p), k, n], concatenated along cols
        xp = np.empty((128, KH * CHUNK), dtype=bf16)
        for j in range(NJ):
            L, o = CHUNKS[j], OFFS[j]
            xp[:, KH * o:KH * (o + L)] = (
                blk[o:o + L, :].reshape(L, KH, 128).transpose(2, 1, 0)
                .reshape(128, KH * L))
        in_maps.append({"xb": xp, "wv": wv_sb, "wo": wo_sb})
    return in_maps


def _run(inputs, trace=False, trace_kwargs=None):
    from concourse import bass_utils

    in_maps = _prep(inputs)
    nc = _get_nc()

    res = bass_utils.run_bass_kernel_spmd(
        nc, in_maps, core_ids=list(range(NCORES)),
        trace=trace, **(trace_kwargs or {}))

    out = np.empty((B, S, H), dtype=np.float32)
    for core in range(NCORES):
        b, q = divmod(core, 4)
        r = res.results[core]["out"]                       # [128, 12288] bf16
        dst = out[b, q * CHUNK:(q + 1) * CHUNK, :]
        for j in range(NJ):
            L, o = CHUNKS[j], OFFS[j]
            dst[o:o + L, :] = (
                r[:, KH * o:KH * (o + L)].reshape(128, KH, L)
                .transpose(2, 1, 0).reshape(L, H).astype(np.float32))
    return out, res


# revision 34
# speedup vs baseline: 1.0175x; 1.0175x over previous
"""Trainium2 Bass kernel for nn_LlamaAttention_7352984010786.

Key insight: the reference's attention matrix is softmax(r @ r.T) where r is
the (input-independent) RoPE sinusoid table.  r_i . r_j = sum_d cos((i-j)*f_d)
is Toeplitz and decays so fast off-diagonal that after softmax the matrix is
numerically tridiagonal:

    probs[i,j] = c_|i-j| / Z_i,   c0 = 1, c1 = 3.6078e-05, c2 = 2.2e-16

So   out = ((v + c1*(v_shift_left + v_shift_right)) / Z) @ Wo.T,  v = x @ Wv.T

The band stencil is a sequence-space convolution and Wv acts on the feature
dim, so they commute:  band(x @ Wv.T) = band(x) @ Wv.T.  The band (and the
1/Z row normalization, folded into Wv with a 2-row edge fixup on x) is applied
ON THE HOST.  The device kernel is then two clean back-to-back matmuls per
core in bf16 (fp32 PSUM accumulation):

    y   = xb @ (Wv.T/Z)      [2048 x 768] @ [768 x 256]
    out = y @ Wo.T           [2048 x 256] @ [256 x 768]

Sharding: batch x seq chunks, 8 cores x 2048 rows.  All tensors are
pre-packed on the host into exact SBUF layout ([128 partitions, free]) so
every DMA moves multi-KB contiguous rows per partition at HBM line rate.
Per-core traffic ~7.1 MB bf16 (~20 us at 358 GB/s) ~= PE time (49k cycles
~= 21 us at 2.4 GHz): the roofline ridge.

Schedule notes (from perfetto traces): each dma_start costs ~0.65 us of
serial descriptor-gen on the issuing engine and each DMA-completion
semaphore takes ~2 us to reach its consumer, so the first weight/input
transfers are split in halves to overlap payload with compute startup; a
memset-fed PE warmup keeps the tensor engine busy (and its DVFS clock
ramped) while the first input chunk lands; PSUM rotates through six
single-bank slots shared by both matmul stages; every output third is
stored as soon as its two copies land so payloads stream during compute;
and the last chunk's final two output pieces take parallel DGE paths
(ACT's own ring + the then-idle Sync ring) to shorten the drain.
"""

import os
import sys

import numpy as np

for _p in ("/opt/trn_rl_repo", os.path.expanduser("~/.axon_site/_ro/trn_rl_repo")):
    if os.path.isdir(_p) and _p not in sys.path:
        sys.path.insert(0, _p)

B, S, H, C = 2, 8192, 768, 256
THETA = 10000.0
NCORES = 8
CHUNK = S // 4      # 2048 seq rows per core; core k: batch k//4, quarter k%4
KH = H // 128       # 6 x-feature blocks (contraction of v-proj)
KC = C // 128       # 2 v-feature blocks (contraction of o-proj)
CHUNKS = [512, 512, 512, 512]               # per-core sequence chunking
OFFS = [0, 512, 1024, 1536]
NJ = len(CHUNKS)

_cache: dict = {}


def _band_constants():
    """c1 and the softmax row-normalizers, in fp64."""
    freqs = THETA ** (-np.arange(0, H, 2, dtype=np.float64) / H)
    dd = np.arange(S, dtype=np.float64)
    g = np.cos(np.outer(dd, freqs)).sum(1)
    e = np.exp(g - g[0])
    c1 = e[1]
    efull = np.concatenate([e[::-1], e[1:]])
    csum = np.concatenate([[0.0], np.cumsum(efull)])
    idx = np.arange(S)
    z = csum[idx + S] - csum[idx]  # Z_i = sum_j e(|i-j|)
    return c1, z


def _build_bass():
    import concourse.bass as bass
    import concourse.tile as tile
    from concourse import bacc, mybir

    f32 = mybir.dt.float32
    bf16 = mybir.dt.bfloat16

    nc = bacc.Bacc("TRN2", target_bir_lowering=False, debug=False,
                   num_devices=NCORES)

    xb_d = nc.dram_tensor("xb", [128, KH * CHUNK], bf16,
                          kind="ExternalInput").ap()
    wv_d = nc.dram_tensor("wv", [128, KH * C], bf16, kind="ExternalInput").ap()
    wo_d = nc.dram_tensor("wo", [128, KC * H], bf16, kind="ExternalInput").ap()
    out_d = nc.dram_tensor("out", [128, KH * CHUNK], bf16,
                           kind="ExternalOutput").ap()

    with tile.TileContext(nc) as tc:
        with (
            tc.tile_pool(name="const", bufs=1) as const_pool,
            tc.tile_pool(name="xin", bufs=NJ) as xin_pool,
            tc.tile_pool(name="y", bufs=1) as y_pool,
            tc.tile_pool(name="outs", bufs=2) as out_pool,
            tc.tile_pool(name="ps", bufs=2, space="PSUM") as ps_pool,
        ):
            # PE warmup fed from a memset tile (no DMA dependency): keeps
            # the tensor engine busy from ~7.5 us so the DVFS clock is
            # fully ramped when the first real matmul issues.  Uses the
            # psv pool (slot is recycled before the first real v-proj).
            wtile = const_pool.tile([128, 512], bf16, name="warm")
            nc.gpsimd.memset(wtile[:], 0.25)
            wps = ps_pool.tile([128, 512], f32, name="ps0")
            for _ in range(10):
                nc.tensor.matmul(wps[:], wtile[:, :128], wtile[:],
                                 start=True, stop=True)
            for _ in range(3):
                nc.tensor.matmul(wps[:, :128], wtile[:, :128],
                                 wtile[:, :128], start=True, stop=True)
            # weights split in k-halves so v-proj(0) can begin after only
            # half the weight bytes + half of chunk 0 have landed
            wv = [const_pool.tile([128, 3 * C], bf16, name=f"wv{i}")
                  for i in range(2)]
            wo = const_pool.tile([128, KC * H], bf16, name="wo")
            xb = [xin_pool.tile([128, KH * CHUNKS[j]], bf16, name=f"xb{j}")
                  for j in range(NJ)]

            # DMA issue order chosen so payloads land just-in-time
            nc.sync.dma_start(wv[0][:], wv_d[:, :3 * C])
            half0 = 3 * CHUNKS[0]
            nc.sync.dma_start(xb[0][:, :half0], xb_d[:, :half0])
            nc.sync.dma_start(xb[0][:, half0:], xb_d[:, half0:2 * half0])
            nc.sync.dma_start(wv[1][:], wv_d[:, 3 * C:])
            half1 = 3 * CHUNKS[1]
            base1 = KH * OFFS[1]
            nc.sync.dma_start(xb[1][:, :half1],
                              xb_d[:, base1:base1 + half1])
            nc.sync.dma_start(xb[1][:, half1:],
                              xb_d[:, base1 + half1:base1 + 2 * half1])
            nc.sync.dma_start(wo[:], wo_d)
            for j in range(2, NJ):
                nc.sync.dma_start(
                    xb[j][:],
                    xb_d[:, KH * OFFS[j]:KH * (OFFS[j] + CHUNKS[j])])

            yT = [y_pool.tile([128, CHUNK], bf16, name=f"y{cs}")
                  for cs in range(KC)]

            def wv_blk(k, cs):
                return wv[k // 3][:, (k % 3) * C + cs * 128:
                                  (k % 3) * C + (cs + 1) * 128]

            def vproj(j):
                L = CHUNKS[j]
                for cs in range(KC):
                    ps = ps_pool.tile([128, L], f32, name=f"ps{cs}")
                    for k in range(KH):
                        nc.tensor.matmul(
                            ps[:], wv_blk(k, cs),
                            xb[j][:, k * L:(k + 1) * L],
                            start=(k == 0), stop=(k == KH - 1),
                        )
                    dst = yT[cs][:, OFFS[j]:OFFS[j] + L]
                    if cs == 0:
                        nc.vector.tensor_copy(dst, ps[:])
                    else:
                        nc.scalar.copy(dst, ps[:])

            onalloc = [0]

            def oproj(j):
                L = CHUNKS[j]
                lo = KH * OFFS[j]
                last = (j == NJ - 1)
                ot = out_pool.tile([128, KH * L], bf16, tag="out",
                                   name=f"ot{j % 2}")
                for hh in range(KH):
                    ps = ps_pool.tile([128, L], f32,
                                      name=f"ps{onalloc[0] % 3}")
                    onalloc[0] += 1
                    for cs in range(KC):
                        nc.tensor.matmul(
                            ps[:],
                            wo[:, cs * H + hh * 128:cs * H + (hh + 1) * 128],
                            yT[cs][:, OFFS[j]:OFFS[j] + L],
                            start=(cs == 0), stop=(cs == KC - 1),
                        )
                    dst = ot[:, hh * L:(hh + 1) * L]
                    if last and hh >= 4:
                        # drain-critical pieces: copy AND issue the store
                        # from parallel DGE paths (ACT's own + idle Sync)
                        if hh == 4:
                            nc.scalar.copy(dst, ps[:])
                            nc.scalar.dma_start(
                                out_d[:, lo + 4 * L:lo + 5 * L], dst)
                        else:
                            nc.vector.tensor_copy(dst, ps[:])
                            nc.sync.dma_start(
                                out_d[:, lo + 5 * L:lo + 6 * L], dst)
                    elif hh % 2 == 0:
                        nc.vector.tensor_copy(dst, ps[:])
                    else:
                        nc.scalar.copy(dst, ps[:])
                        # store each third as soon as its copies land
                        nc.sync.dma_start(
                            out_d[:, lo + (hh - 1) * L:lo + (hh + 1) * L],
                            ot[:, (hh - 1) * L:(hh + 1) * L])

            vproj(0)
            for j in range(NJ):
                if j + 1 < NJ:
                    vproj(j + 1)
                oproj(j)

    nc.compile()
    return nc


def _get_nc():
    if "nc" not in _cache:
        _cache["nc"] = _build_bass()
    return _cache["nc"]


def kernel(**inputs) -> np.ndarray:
    out, _ = _run(inputs)
    return out


def _prep(inputs):
    import ml_dtypes
    bf16 = ml_dtypes.bfloat16

    x = np.asarray(inputs["x"], dtype=np.float32)
    Wv = np.asarray(inputs["Wv"], dtype=np.float32)
    Wo = np.asarray(inputs["Wo"], dtype=np.float32)

    c1, z = _band_constants()
    z_int = 1.0 + 2.0 * c1

    # band on x (commutes with the projections); 1/Z folded into Wv as
    # 1/z_int, with the two edge rows rescaled here to their true Z.
    xb = np.empty_like(x)
    xb[:, 1:-1, :] = x[:, 1:-1, :] + np.float32(c1) * (x[:, :-2, :]
                                                       + x[:, 2:, :])
    xb[:, 0, :] = (x[:, 0, :] + np.float32(c1) * x[:, 1, :]) \
        * np.float32(z_int / z[0])
    xb[:, -1, :] = (x[:, -1, :] + np.float32(c1) * x[:, -2, :]) \
        * np.float32(z_int / z[-1])
    xb = xb.astype(bf16)

    # SBUF-layout packing: wv[p, k, c] = Wv[c, k*128+p]/z_int
    wv_sb = np.ascontiguousarray(
        (Wv.T * np.float32(1.0 / z_int)).astype(bf16)
        .reshape(KH, 128, C).transpose(1, 0, 2).reshape(128, KH * C))
    # wo[p, s, h] = Wo[h, s*128+p]
    wo_sb = np.ascontiguousarray(
        Wo.T.astype(bf16).reshape(KC, 128, H).transpose(1, 0, 2)
        .reshape(128, KC * H))

    in_maps = []
    for core in range(NCORES):
        b, q = divmod(core, 4)
        blk = xb[b, q * CHUNK:(q + 1) * CHUNK, :]          # [2048, 768]
        # per chunk j: [L, 768] -> [128(p), k, n], concatenated along cols
        xp = np.empty((128, KH * CHUNK), dtype=bf16)
        for j in range(NJ):
            L, o = CHUNKS[j], OFFS[j]
            xp[:, KH * o:KH * (o + L)] = (
                blk[o:o + L, :].reshape(L, KH, 128).transpose(2, 1, 0)
                .reshape(128, KH * L))
        in_maps.append({"xb": xp, "wv": wv_sb, "wo": wo_sb})
    return in_maps


def _run(inputs, trace=False, trace_kwargs=None):
    from concourse import bass_utils

    in_maps = _prep(inputs)
    nc = _get_nc()

    res = bass_utils.run_bass_kernel_spmd(
        nc, in_maps, core_ids=list(range(NCORES)),
        trace=trace, **(trace_kwargs or {}))

    out = np.empty((B, S, H), dtype=np.float32)
    for core in range(NCORES):
        b, q = divmod(core, 4)
        r = res.results[core]["out"]                       # [128, 12288] bf16
        dst = out[b, q * CHUNK:(q + 1) * CHUNK, :]
        for j in range(NJ):
            L, o = CHUNKS[j], OFFS[j]
            dst[o:o + L, :] = (
                r[:, KH * o:KH * (o + L)].reshape(128, KH, L)
                .transpose(2, 1, 0).reshape(L, H).astype(np.float32))
    return out, res
